# revision 1
# baseline (speedup 1.0000x reference)
"""GPSA (gated positional self-attention) Trainium2 Bass kernel.

Problem: B=16, N=576, C=768, H=12 heads, hd=64.
  qk = x @ Wqk.T -> q,k [B,H,N,64]
  patch = softmax(q k^T / 8), pos = softmax(rel @ Wpos + bpos)  [H,N,N]
  attn = (1-sig(g))*patch + sig(g)*pos  (sums to 1 -> final renorm is identity)
  out = (attn @ v) @ Wproj.T + bproj

Sharding: pure data-parallel over batch, 2 batches per core, no collectives.

Cost-model-driven layout (matmul cost = out-free-size x k-tiles, fp8 DoubleRow
halves it again by packing 2 k-slabs per instruction at 0.5 cyc/row):
  qk/v projections run in fp8e4 with a hi/lo 3-term split (W*64 host-scaled to
  keep the lo residual out of the fp8 subnormal range; the 64x is compensated
  in the exp scale, the host-prescaled pposT, and the Z ones-column).
  S^T[m,n] = k-stationary matmul vs q (K=64, heads row-packed via
  tile_position); exp on ACT (scale=1/(8*4096); max-subtraction skipped).
  PV runs "flipped": es/ppos [m, n-chunk<=128] stationary, v moving [m, 65]
  -> out [n-chunk, 65] per chunk: 64-col outputs per chunk instead of
  576-col ones (1600+1625 vs 5760 cycles/item). Column 64 of the content
  output is the softmax normalizer Z' = WS/(1-g) * Z via a constant column
  appended to v, so the combine is per-partition-scalar work on DVE:
  o[n,c] = pos + content * recip(Z'col).
  o[n,c] is PE-transposed (identity trick) into oT[c,n] and projected
  W-stationary in bf16.
PSUM: chains (qk/V/proj) use 3x1-bank [128,512] tiles, S^T 2x2-bank
[128,576], PV one 1-bank [128,3,129] two-pass tile -> exactly 8 banks with
no contention between the S/exp pipeline and the projection chains.
The host precomputes sigmoid(gating), the positional softmax (depends only on
Wpos/bpos/gating; prescaled by g/WS), all weight transposes and fp8 splits.
"""

import numpy as np
import ml_dtypes

from contextlib import ExitStack

import concourse.tile as tile
from concourse import bacc, mybir
from concourse.bass_utils import run_bass_kernel_spmd

BF16 = mybir.dt.bfloat16
F32 = mybir.dt.float32
FP8 = mybir.dt.float8e4
AF = mybir.ActivationFunctionType
ALU = mybir.AluOpType
DR = mybir.MatmulPerfMode.DoubleRow

B, N, C, H = 16, 576, 768, 12
HD = C // H                      # 64
NCORES = 8
BLOC = B // NCORES               # batches per core
NT = BLOC * N                    # tokens per core (1152)
F = 2 * C                        # fused qk features (1536)
KT = C // 128                    # 6 contraction tiles over C
FT = F // 128                    # 12 feature tiles of qkT
MT = (N + 127) // 128            # 5 m-tiles per batch (last has 64 rows)
WS = 64.0                        # host pre-scale on Wqk/Wv for fp8 hi/lo
EXPSCALE = (HD ** -0.5) / (WS * WS)   # 0.125 / 4096


def _mrows(mt):
    return min(128, N - mt * 128)


def build_program(reps=1):
    nc = bacc.Bacc("TRN2", target_bir_lowering=False, debug=False,
                   num_devices=NCORES)

    xh = nc.declare_dram_parameter("xh", [C, NT], FP8, isOutput=False)
    xl = nc.declare_dram_parameter("xl", [C, NT], FP8, isOutput=False)
    wqh = nc.declare_dram_parameter("wqh", [C, F], FP8, isOutput=False)
    wql = nc.declare_dram_parameter("wql", [C, F], FP8, isOutput=False)
    wvh = nc.declare_dram_parameter("wvh", [C, C], FP8, isOutput=False)
    wvl = nc.declare_dram_parameter("wvl", [C, C], FP8, isOutput=False)
    wprojT = nc.declare_dram_parameter("wprojT", [C, C], BF16, isOutput=False)
    ident = nc.declare_dram_parameter("ident", [128, 128], BF16, isOutput=False)
    vones = nc.declare_dram_parameter("vones", [128, BLOC * MT * H], F32,
                                      isOutput=False)
    pposT = nc.declare_dram_parameter("pposT", [H, N, N], BF16, isOutput=False)
    yT = nc.declare_dram_parameter("yT", [BLOC, C, N], F32, isOutput=True)

    with tile.TileContext(nc) as tc, ExitStack() as ctx:
        sbw = ctx.enter_context(tc.tile_pool(name="sbw", bufs=1))
        sbact = ctx.enter_context(tc.tile_pool(name="sbact", bufs=1))
        ppos_pool = ctx.enter_context(tc.tile_pool(name="ppos", bufs=5))
        es_pool = ctx.enter_context(tc.tile_pool(name="es", bufs=6))
        zr_pool = ctx.enter_context(tc.tile_pool(name="zr", bufs=4))
        y_pool = ctx.enter_context(tc.tile_pool(name="ysb", bufs=4))

        # ---- constants / weights (qk inputs first for an early exp start;
        # b0 halves of x first so the first qk chains complete sooner) ----
        xh_sb = sbw.tile([128, KT, NT], FP8)
        xl_sb = sbw.tile([128, KT, NT], FP8)
        wqh_sb = sbw.tile([128, KT, F], FP8)
        wql_sb = sbw.tile([128, KT, F], FP8)
        wvh_sb = sbw.tile([128, KT, C], FP8)
        wvl_sb = sbw.tile([128, KT, C], FP8)
        wprojT_sb = sbw.tile([128, KT, C], BF16)
        ident_sb = sbw.tile([128, 128], BF16)
        vones_sb = sbw.tile([128, BLOC * MT * H], F32)
        for k in range(KT):
            ks = slice(k * 128, (k + 1) * 128)
            nc.sync.dma_start(xh_sb[:, k, 0:N], xh[ks, 0:N])
            nc.sync.dma_start(wqh_sb[:, k, :], wqh[ks, :])
            nc.sync.dma_start(wql_sb[:, k, :], wql[ks, :])
            nc.sync.dma_start(xl_sb[:, k, 0:N], xl[ks, 0:N])
        for k in range(KT):
            ks = slice(k * 128, (k + 1) * 128)
            nc.sync.dma_start(xh_sb[:, k, N:NT], xh[ks, N:NT])
            nc.sync.dma_start(xl_sb[:, k, N:NT], xl[ks, N:NT])
        for k in range(KT):
            ks = slice(k * 128, (k + 1) * 128)
            nc.sync.dma_start(wvh_sb[:, k, :], wvh[ks, :])
            nc.sync.dma_start(wvl_sb[:, k, :], wvl[ks, :])
        nc.sync.dma_start(ident_sb[:], ident[:, :])
        nc.sync.dma_start(vones_sb[:], vones[:, :])
        for k in range(KT):
            nc.sync.dma_start(wprojT_sb[:, k, :], wprojT[k * 128:(k + 1) * 128, :])

        for _ in range(reps):
            ps_a_ctx = tc.tile_pool(name="ps_a", bufs=3, space="PSUM")
            psA = ps_a_ctx.__enter__()
            ps_s_ctx = tc.tile_pool(name="ps_s", bufs=2, space="PSUM")
            psS = ps_s_ctx.__enter__()
            ps_cp_ctx = tc.tile_pool(name="ps_cp", bufs=1, space="PSUM")
            psCP = ps_cp_ctx.__enter__()

            qkT_sb = sbact.tile([128, FT, NT], BF16)
            # v layout [tokens, 12*65]: per head 64 v-cols + a constant column
            # WS/(1-g_h) making the content-PV's 65th output column the
            # prescaled softmax normalizer Z'
            vv_sb = sbact.tile([128, BLOC, MT, H * (HD + 1)], BF16)
            ones_cols = vv_sb[:, :, :, :].rearrange(
                "p b m (h e) -> p b m h e", e=HD + 1)[:, :, :, :, HD:HD + 1]
            vones_src = vones_sb[:, :].rearrange(
                "p (b m h) -> p b m h", b=BLOC, m=MT)[:, :, :, :, None]
            nc.vector.tensor_copy(ones_cols, vones_src)
            # o[n, c] per batch: [n-part within chunk, b, n-chunk, (h d)]
            o_sb = sbact.tile([128, BLOC, MT, C], BF16)
            oT_sb = sbact.tile([128, KT, NT], BF16)

            items = [(hp, b, hsub)
                     for hp in range(H // 2)
                     for b in range(BLOC)
                     for hsub in range(2)]
            ppos_tiles = {}
            es_tiles = {}

            def dma_ppos(hp):
                for hsub in range(2):
                    h = 2 * hp + hsub
                    pt = ppos_pool.tile([128, MT, N], BF16, tag="ppos")
                    for mt in range(MT):
                        mr = _mrows(mt)
                        nc.sync.dma_start(
                            pt[:mr, mt, :], pposT[h, mt * 128:mt * 128 + mr, :])
                    ppos_tiles[h] = pt

            def prepare(i):
                hp, b, hsub = items[i]
                h = 2 * hp + hsub
                if b == 0 and hsub == 0:
                    dma_ppos(hp)
                qrows = slice((h % 2) * 64, (h % 2) * 64 + 64)
                qt, kt_ = h // 2, FT // 2 + h // 2
                es = es_pool.tile([128, MT, N], BF16, tag="es")
                for mt in range(MT):
                    mr = _mrows(mt)
                    p_s = psS.tile([128, N], F32, tag="S")
                    for (no, nw) in ((0, 512), (512, 64)):
                        nc.tensor.matmul(
                            p_s[:mr, no:no + nw],
                            qkT_sb[qrows, kt_,
                                   b * N + mt * 128: b * N + mt * 128 + mr],
                            qkT_sb[qrows, qt, b * N + no: b * N + no + nw],
                            start=True, stop=True,
                            tile_position=((h % 2) * 64, 0))
                    nc.scalar.activation(
                        es[:mr, mt, :], p_s[:mr, 0:N], AF.Exp, scale=EXPSCALE)
                es_tiles[i] = es

            def consume(i):
                hp, b, hsub = items[i]
                h = 2 * hp + hsub
                es = es_tiles.pop(i)
                hc = slice(h * (HD + 1), h * (HD + 1) + HD)
                hc1 = slice(h * (HD + 1), (h + 1) * (HD + 1))
                oh = slice(h * HD, (h + 1) * HD)
                for chunks in ((0, 1, 2), (3, 4)):
                    cp = psCP.tile([128, 3, 2 * HD + 1], F32, tag="cp")
                    for ci, c in enumerate(chunks):
                        nr = _mrows(c)
                        for mt in range(MT):
                            mr = _mrows(mt)
                            nc.tensor.matmul(
                                cp[:nr, ci, HD + 1:2 * HD + 1],
                                ppos_tiles[h][:mr, mt, c * 128:c * 128 + nr],
                                vv_sb[:mr, b, mt, hc],
                                start=(mt == 0), stop=(mt == MT - 1))
                        for mt in range(MT):
                            mr = _mrows(mt)
                            nc.tensor.matmul(
                                cp[:nr, ci, 0:HD + 1],
                                es[:mr, mt, c * 128:c * 128 + nr],
                                vv_sb[:mr, b, mt, hc1],
                                start=(mt == 0), stop=(mt == MT - 1))
                    # per-chunk normalizers 1/Z' (chunk 4 has 64 valid rows);
                    # evict pos to o, then o = content * (1/Z') + o
                    zr = zr_pool.tile([128, 3, 1], F32, tag="zr")
                    if chunks[0] == 0:
                        nc.vector.reciprocal(zr[:, 0:3, :], cp[:, 0:3, HD:HD + 1])
                        nc.vector.tensor_copy(
                            o_sb[:, b, 0:3, oh], cp[:, 0:3, HD + 1:2 * HD + 1])
                    else:
                        nc.vector.reciprocal(zr[:, 0:1, :], cp[:, 0:1, HD:HD + 1])
                        nc.vector.reciprocal(zr[:64, 1:2, :],
                                             cp[:64, 1:2, HD:HD + 1])
                        nc.vector.tensor_copy(
                            o_sb[:, b, 3, oh], cp[:, 0, HD + 1:2 * HD + 1])
                        nc.vector.tensor_copy(
                            o_sb[:64, b, 4, oh], cp[:64, 1, HD + 1:2 * HD + 1])
                    for ci, c in enumerate(chunks):
                        nr = _mrows(c)
                        nc.vector.scalar_tensor_tensor(
                            o_sb[:nr, b, c, oh], cp[:nr, ci, 0:HD],
                            zr[:nr, ci, 0:1], o_sb[:nr, b, c, oh],
                            op0=ALU.mult, op1=ALU.add)

            TERMS_QK = ((wqh_sb, xh_sb), (wql_sb, xh_sb), (wqh_sb, xl_sb))
            TERMS_V = ((xh_sb, wvh_sb), (xh_sb, wvl_sb), (xl_sb, wvh_sb))

            def qk_chain(fc, b):
                fs = slice(fc * 128, (fc + 1) * 128)
                tA = psA.tile([128, 512], F32, tag="A")
                tB = psA.tile([128, 512], F32, tag="A")
                for (no, nw, t, off) in ((0, 256, tA, 0), (256, 256, tA, 256),
                                         (512, 64, tB, 0)):
                    nmm = 0
                    for (w8, x8) in TERMS_QK:
                        for j in range(KT // 2):
                            nc.tensor.matmul(
                                t[:, off:off + nw],
                                w8[:, 2 * j:2 * j + 2, fs],
                                x8[:, 2 * j:2 * j + 2,
                                   b * N + no: b * N + no + nw],
                                start=(nmm == 0), stop=(nmm == 8),
                                perf_mode=DR)
                            nmm += 1
                nc.vector.tensor_copy(
                    qkT_sb[:, fc, b * N:b * N + 512], tA[:, 0:512])
                nc.vector.tensor_copy(
                    qkT_sb[:, fc, b * N + 512:(b + 1) * N], tB[:, 0:64])

            def v_chain(b, mt):
                mr = _mrows(mt)
                ms = slice(b * N + mt * 128, b * N + mt * 128 + mr)
                tA = psA.tile([128, 512], F32, tag="A")
                tB = psA.tile([128, 512], F32, tag="A")
                for (co, t, off) in ((0, tA, 0), (256, tA, 256), (512, tB, 0)):
                    nmm = 0
                    for (x8, w8) in TERMS_V:
                        for j in range(KT // 2):
                            nc.tensor.matmul(
                                t[:mr, off:off + 256],
                                x8[:, 2 * j:2 * j + 2, ms],
                                w8[:, 2 * j:2 * j + 2, co:co + 256],
                                start=(nmm == 0), stop=(nmm == 8),
                                perf_mode=DR)
                            nmm += 1
                vvr = vv_sb[:mr, b, mt, :].rearrange(
                    "p (h e) -> p h e", e=HD + 1)[:, :, 0:HD]
                nc.vector.tensor_copy(
                    vvr[:, 0:8, :],
                    tA[:mr, 0:512].rearrange("p (h e) -> p h e", e=HD))
                nc.vector.tensor_copy(
                    vvr[:, 8:12, :],
                    tB[:mr, 0:256].rearrange("p (h e) -> p h e", e=HD))

            def transpose_block(b, cc):
                # oT psums share the S-psum slots (fits: bf16 576 <= f32 576)
                p_t = psS.tile([128, N], BF16, tag="S")
                for nt in range(MT):
                    nr = _mrows(nt)
                    nc.tensor.transpose(
                        p_t[:, nt * 128:nt * 128 + nr],
                        o_sb[:nr, b, nt, cc * 128:(cc + 1) * 128],
                        ident_sb[:nr, :nr])
                nc.vector.tensor_copy(
                    oT_sb[:, cc, b * N:(b + 1) * N], p_t[:, 0:N])

            def proj(b):
                for cc in range(KT):
                    tA = psA.tile([128, 512], F32, tag="A")
                    tB = psA.tile([128, 512], F32, tag="A")
                    for k in range(KT):
                        for (no, nw, t, off) in ((0, 512, tA, 0),
                                                 (512, 64, tB, 0)):
                            nc.tensor.matmul(
                                t[:, off:off + nw],
                                wprojT_sb[:, k, cc * 128:(cc + 1) * 128],
                                oT_sb[:, k, b * N + no: b * N + no + nw],
                                start=(k == 0), stop=(k == KT - 1))
                    y1 = y_pool.tile([128, 512], F32, tag="ysb")
                    nc.scalar.copy(y1[:], tA[:, 0:512])
                    nc.sync.dma_start(
                        yT[b, cc * 128:(cc + 1) * 128, 0:512], y1[:])
                    y2 = y_pool.tile([128, 512], F32, tag="ysb")
                    nc.scalar.copy(y2[:, 0:64], tB[:, 0:64])
                    nc.sync.dma_start(
                        yT[b, cc * 128:(cc + 1) * 128, 512:576], y2[:, 0:64])

            # ---- head start: hp0's q/k chains + preps so ACT (exp) starts
            # as early as possible; its exps overlap the v projection ----
            qk_chain(0, 0)
            qk_chain(FT // 2, 0)
            prepare(0)
            prepare(1)
            qk_chain(0, 1)
            qk_chain(FT // 2, 1)
            prepare(2)
            prepare(3)

            # ---- v[m, c'] (x-stationary, fp8 hi/lo DoubleRow) ----
            for b in range(BLOC):
                for mt in range(MT):
                    v_chain(b, mt)

            # ---- item-driven pipeline: chains just-in-time per (hp, b),
            # preps paced 1:1 with consumes so S/exp work fills PE/ACT until
            # the very end; transposes/projection interleave as (hp, b)
            # blocks complete ----
            LAG = 4
            for i in range(LAG, len(items) + LAG):
                if i < len(items):
                    hp, b, hsub = items[i]
                    if hsub == 0 and hp >= 1:
                        qk_chain(hp, b)
                        qk_chain(FT // 2 + hp, b)
                    prepare(i)
                hp_c, b_c, hsub_c = items[i - LAG]
                consume(i - LAG)
                if hsub_c == 1:
                    transpose_block(b_c, hp_c)
                    if hp_c == H // 2 - 1:
                        proj(b_c)
            ps_cp_ctx.__exit__(None, None, None)
            ps_s_ctx.__exit__(None, None, None)
            ps_a_ctx.__exit__(None, None, None)

    nc.compile()
    return nc


_CACHE = {}


def _get_program(reps=1):
    if reps not in _CACHE:
        _CACHE[reps] = build_program(reps)
    return _CACHE[reps]


def _fp8_hilo(w):
    fp8 = ml_dtypes.float8_e4m3
    hi = w.astype(np.float32).astype(fp8)
    lo = (w.astype(np.float32) - hi.astype(np.float32)).astype(fp8)
    return hi, lo


def _host_prep(x, Wqk, Wv, Wproj, bproj, Wpos, bpos, gating):
    bf = ml_dtypes.bfloat16
    g = 1.0 / (1.0 + np.exp(-gating.astype(np.float64)))          # [H]

    s = int(N ** 0.5)
    ind = np.arange(s)[None, :] - np.arange(s)[:, None]
    indx = np.tile(ind, (s, s)).astype(np.float64)
    indy = np.repeat(np.repeat(ind, s, axis=0), s, axis=1).astype(np.float64)
    indd = indx ** 2 + indy ** 2
    rel = np.stack([indx, indy, indd], axis=-1)                    # [N, N, 3]
    pos_logits = np.einsum("nmt,ht->hnm", rel, Wpos.astype(np.float64))
    pos_logits += bpos.astype(np.float64)[:, None, None]
    pos_logits -= pos_logits.max(axis=-1, keepdims=True)
    e = np.exp(pos_logits)
    pos = e / e.sum(axis=-1, keepdims=True)                        # [H, n, m]
    # prescaled by g/WS to undo the WS factor the fp8 v projection carries
    ppos_w = (g[:, None, None] / WS) * pos
    pposT = np.ascontiguousarray(ppos_w.transpose(0, 2, 1)).astype(bf)

    wqh, wql = _fp8_hilo(np.ascontiguousarray(Wqk.T) * WS)
    wvh, wvl = _fp8_hilo(np.ascontiguousarray(Wv.T) * WS)
    # Z' column value WS/(1-g): combine's recip then yields (1-g)/(WS*Z)
    # so content * that + pos is exact
    vcol = (WS / (1.0 - g)).astype(np.float32)                     # [H]
    vones = np.broadcast_to(vcol[None, None, :],
                            (BLOC * MT, 128, H)).transpose(1, 0, 2)
    vones = np.ascontiguousarray(vones.reshape(128, BLOC * MT * H))

    common = {
        "wqh": wqh, "wql": wql, "wvh": wvh, "wvl": wvl,
        "wprojT": np.ascontiguousarray(Wproj.T).astype(bf),
        "ident": np.eye(128, dtype=np.float32).astype(bf),
        "vones": vones,
        "pposT": pposT,
    }
    in_maps = []
    for i in range(NCORES):
        xc = x[i * BLOC:(i + 1) * BLOC]                            # [2, 576, 768]
        xTl = np.ascontiguousarray(
            xc.transpose(2, 0, 1).reshape(C, NT)).astype(np.float32)
        xhl, xll = _fp8_hilo(xTl)
        in_maps.append({"xh": xhl, "xl": xll, **common})
    return in_maps


def kernel(x, Wqk, Wv, Wproj, bproj, Wpos, bpos, gating):
    x = np.asarray(x, dtype=np.float32)
    in_maps = _host_prep(np.asarray(x, np.float32), np.asarray(Wqk, np.float32),
                         np.asarray(Wv, np.float32), np.asarray(Wproj, np.float32),
                         np.asarray(bproj, np.float32), np.asarray(Wpos, np.float32),
                         np.asarray(bpos, np.float32), np.asarray(gating, np.float32))
    nc = _get_program(reps=1)
    res = run_bass_kernel_spmd(nc, in_maps, list(range(NCORES)))
    outs = []
    for i in range(NCORES):
        yTl = res.results[i]["yT"]                                 # [2, 768, 576]
        outs.append(yTl.transpose(0, 2, 1))                        # [2, 576, 768]
    out = np.concatenate(outs, axis=0).astype(np.float32)
    out += np.asarray(bproj, np.float32)[None, None, :]
    return np.ascontiguousarray(out)



# revision 4
# speedup vs baseline: 1.1288x; 1.1288x over previous
"""GPSA (gated positional self-attention) Trainium2 Bass kernel.

Problem: B=16, N=576, C=768, H=12 heads, hd=64.
  qk = x @ Wqk.T -> q,k [B,H,N,64]
  patch = softmax(q k^T / 8), pos = softmax(rel @ Wpos + bpos)  [H,N,N]
  attn = (1-sig(g))*patch + sig(g)*pos  (sums to 1 -> final renorm is identity)
  out = (attn @ v) @ Wproj.T + bproj

Sharding: pure data-parallel over batch, 2 batches per core, no collectives.

Cost-model-driven layout (matmul cost = out-free-size x k-instrs; fp8
DoubleRow halves the per-instr cost by packing 2 contraction slabs):
  qk projection: fp8 2-term (Wqk hi/lo at WSQ=32, x hi only) -> psum holds
  WSQ*q; evicted directly to fp8 q/k (|WSQ*q| < 240).
  Head features are host-permuted so head h (quad P=h//4, slot s=h%4) has
  q features as [32 partitions at 32s, 2 planes = chunks 4P/4P+1] and k in
  chunks 4P+2/4P+3.  S^T then runs as a DoubleRow fp8 matmul (0.5 cyc/row):
  stationary k [32, 2, m], moving q [32, 2, n], tile_position (32s, 0).
  v projection: fp8 3-term hi/lo at WS=64 (v feeds the output linearly, so
  it needs the extra term; q/k errors wash out through softmax).
  exp on ACT (scale 1/(8*WSQ^2); max-subtraction skipped).
  PV runs "flipped" in bf16: es/ppos [m, n-chunk<=128] stationary, v moving
  [m, 65] -> out [n-chunk, 65].  Column 64 of the content output is the
  softmax normalizer Z' = WS/(1-g) * Z via a constant column appended to v.
  The combine is a single fused DVE op per chunk:
  o[n,c] = content * recip(Z'col) + pos  (both read straight from PSUM).
  o[n,c] is PE-transposed (identity trick) into oT[c,n] and projected
  W-stationary in bf16.
PSUM: chains (qk/V/proj) use 3x1-bank [128,512] tiles, S^T 2x2-bank
[128,576], PV one 1-bank [128,3,129] two-pass tile -> exactly 8 banks.
DMA: one large transfer per tensor region (HWDGE generation is 625ns per
DMA, serialized), ordered so the first qk chains / v chains / ppos arrive
just in time.  The host precomputes sigmoid(gating), the positional softmax
(prescaled by g/WS), weight transposes, permutations and fp8 splits.
"""

import numpy as np
import ml_dtypes

from contextlib import ExitStack

import concourse.tile as tile
from concourse import bacc, mybir
from concourse.bass_utils import run_bass_kernel_spmd

BF16 = mybir.dt.bfloat16
F32 = mybir.dt.float32
FP8 = mybir.dt.float8e4
AF = mybir.ActivationFunctionType
ALU = mybir.AluOpType
DR = mybir.MatmulPerfMode.DoubleRow

B, N, C, H = 16, 576, 768, 12
HD = C // H                      # 64
NCORES = 8
BLOC = B // NCORES               # batches per core
NT = BLOC * N                    # tokens per core (1152)
F = 2 * C                        # fused qk features (1536)
KT = C // 128                    # 6 contraction tiles over C
FT = F // 128                    # 12 feature tiles of qkT
MT = (N + 127) // 128            # 5 m-tiles per batch (last has 64 rows)
NQ = 3                           # head quads
WS = 64.0                        # host pre-scale on Wv for fp8 hi/lo
WSQ = 32.0                       # host pre-scale on Wqk (fp8 q/k fits 240)
EXPSCALE = (HD ** -0.5) / (WSQ * WSQ)


def _mrows(mt):
    return min(128, N - mt * 128)


def build_program(reps=1):
    nc = bacc.Bacc("TRN2", target_bir_lowering=False, debug=False,
                   num_devices=NCORES)

    xh = nc.declare_dram_parameter("xh", [C, NT], FP8, isOutput=False)
    xl = nc.declare_dram_parameter("xl", [C, NT], FP8, isOutput=False)
    wqh = nc.declare_dram_parameter("wqh", [C, F], FP8, isOutput=False)
    wql = nc.declare_dram_parameter("wql", [C, F], FP8, isOutput=False)
    wvh = nc.declare_dram_parameter("wvh", [C, C], FP8, isOutput=False)
    wvl = nc.declare_dram_parameter("wvl", [C, C], FP8, isOutput=False)
    wprojT = nc.declare_dram_parameter("wprojT", [C, C], BF16, isOutput=False)
    ident = nc.declare_dram_parameter("ident", [128, 128], BF16, isOutput=False)
    vones = nc.declare_dram_parameter("vones", [128, BLOC * MT * H], F32,
                                      isOutput=False)
    pposT = nc.declare_dram_parameter("pposT", [H, N, N], BF16, isOutput=False)
    yT = nc.declare_dram_parameter("yT", [BLOC, C, N], F32, isOutput=True)

    with tile.TileContext(nc) as tc, ExitStack() as ctx:
        sbw = ctx.enter_context(tc.tile_pool(name="sbw", bufs=1))
        sbact = ctx.enter_context(tc.tile_pool(name="sbact", bufs=1))
        ppos_pool = ctx.enter_context(tc.tile_pool(name="ppos", bufs=5))
        es_pool = ctx.enter_context(tc.tile_pool(name="es", bufs=6))
        zr_pool = ctx.enter_context(tc.tile_pool(name="zr", bufs=4))
        y_pool = ctx.enter_context(tc.tile_pool(name="ysb", bufs=4))

        # ---- weights / constants.  Few large DMAs (HWDGE gen is 625ns
        # each, serialized) ordered so the first qk chains start ASAP. ----
        xh_sb = sbw.tile([128, KT, NT], FP8)
        xl_sb = sbw.tile([128, KT, NT], FP8)
        wqh_sb = sbw.tile([128, KT, F], FP8)
        wql_sb = sbw.tile([128, KT, F], FP8)
        wvh_sb = sbw.tile([128, KT, C], FP8)
        wvl_sb = sbw.tile([128, KT, C], FP8)
        wprojT_sb = sbw.tile([128, KT, C], BF16)
        ident_sb = sbw.tile([128, 128], BF16)
        vones_sb = sbw.tile([128, BLOC * MT * H], F32)

        def _r(dram, cols):          # [rows, cols] -> [128, KT', cols]
            return dram.rearrange("(k p) n -> p k n", p=128)

        xh_r, xl_r = _r(xh, NT), _r(xl, NT)
        wqh_r, wql_r = _r(wqh, F), _r(wql, F)

        def load_wq_quad(q):
            fs = slice(512 * q, 512 * (q + 1))
            nc.sync.dma_start(wqh_sb[:, :, fs], wqh_r[:, :, fs])
            nc.sync.dma_start(wql_sb[:, :, fs], wql_r[:, :, fs])

        # quad 0 weights + batch-0 x: unblocks qk chains 0-3
        nc.sync.dma_start(xh_sb[:, :, 0:N], xh_r[:, :, 0:N])
        load_wq_quad(0)
        nc.sync.dma_start(xl_sb[:, :, 0:N], xl_r[:, :, 0:N])
        # v weights (v chains for b0 follow the first 4 prepares)
        nc.sync.dma_start(wvh_sb[:, :, :], _r(wvh, C)[:, :, :])
        nc.sync.dma_start(wvl_sb[:, :, :], _r(wvl, C)[:, :, :])
        nc.sync.dma_start(xh_sb[:, :, N:NT], xh_r[:, :, N:NT])
        nc.sync.dma_start(xl_sb[:, :, N:NT], xl_r[:, :, N:NT])
        load_wq_quad(1)
        nc.sync.dma_start(ident_sb[:], ident[:, :])
        nc.sync.dma_start(vones_sb[:], vones[:, :])
        load_wq_quad(2)
        nc.sync.dma_start(wprojT_sb[:, :, :],
                          wprojT.rearrange("(k p) n -> p k n", p=128)[:, :, :])

        for _ in range(reps):
            ps_a_ctx = tc.tile_pool(name="ps_a", bufs=3, space="PSUM")
            psA = ps_a_ctx.__enter__()
            ps_s_ctx = tc.tile_pool(name="ps_s", bufs=2, space="PSUM")
            psS = ps_s_ctx.__enter__()
            ps_cp_ctx = tc.tile_pool(name="ps_cp", bufs=1, space="PSUM")
            psCP = ps_cp_ctx.__enter__()

            qkT_sb = sbact.tile([128, FT, NT], FP8)
            # v layout [tokens, 12*65]: per head 64 v-cols + a constant column
            # WS/(1-g_h) making the content-PV's 65th output column the
            # prescaled softmax normalizer Z'
            vv_sb = sbact.tile([128, BLOC, MT, H * (HD + 1)], BF16)
            ones_cols = vv_sb[:, :, :, :].rearrange(
                "p b m (h e) -> p b m h e", e=HD + 1)[:, :, :, :, HD:HD + 1]
            vones_src = vones_sb[:, :].rearrange(
                "p (b m h) -> p b m h", b=BLOC, m=MT)[:, :, :, :, None]
            nc.vector.tensor_copy(ones_cols, vones_src)
            # o[n, c] per batch: [n-part within chunk, b, n-chunk, (h d)]
            o_sb = sbact.tile([128, BLOC, MT, C], BF16)
            oT_sb = sbact.tile([128, KT, NT], BF16)

            items = [(4 * Q + hs, b)
                     for Q in range(NQ)
                     for b in range(BLOC)
                     for hs in range(4)]
            ppos_tiles = {}
            es_tiles = {}

            def dma_ppos(hp):
                for hsub in range(2):
                    h = 2 * hp + hsub
                    pt = ppos_pool.tile([128, MT, N], BF16, tag="ppos")
                    pr = pposT[h, 0:512, :].rearrange("(m p) n -> p m n", p=128)
                    nc.sync.dma_start(pt[:, 0:4, :], pr)
                    nc.sync.dma_start(pt[0:64, 4, :],
                                      pposT[h, 512:576, :])
                    ppos_tiles[h] = pt

            def prepare(i):
                h, b = items[i]
                if b == 0 and h % 2 == 0:
                    dma_ppos(h // 2)
                P, s = h // 4, h % 4
                rows = slice(32 * s, 32 * s + 32)
                qc = slice(4 * P, 4 * P + 2)
                kc = slice(4 * P + 2, 4 * P + 4)
                es = es_pool.tile([128, MT, N], BF16, tag="es")
                for mt in range(MT):
                    mr = _mrows(mt)
                    ms = slice(b * N + mt * 128, b * N + mt * 128 + mr)
                    p_s = psS.tile([128, N], F32, tag="S")
                    for (no, nw) in ((0, 512), (512, 64)):
                        nc.tensor.matmul(
                            p_s[:mr, no:no + nw],
                            qkT_sb[rows, kc, ms],
                            qkT_sb[rows, qc, b * N + no: b * N + no + nw],
                            start=True, stop=True, perf_mode=DR,
                            tile_position=(32 * s, 0))
                    nc.scalar.activation(
                        es[:mr, mt, :], p_s[:mr, 0:N], AF.Exp, scale=EXPSCALE)
                es_tiles[i] = es

            def consume(i):
                h, b = items[i]
                es = es_tiles.pop(i)
                hc = slice(h * (HD + 1), h * (HD + 1) + HD)
                hc1 = slice(h * (HD + 1), (h + 1) * (HD + 1))
                oh = slice(h * HD, (h + 1) * HD)
                for chunks in ((0, 1, 2), (3, 4)):
                    cp = psCP.tile([128, 3, 2 * HD + 1], F32, tag="cp")
                    for ci, c in enumerate(chunks):
                        nr = _mrows(c)
                        for mt in range(MT):
                            mr = _mrows(mt)
                            nc.tensor.matmul(
                                cp[:nr, ci, HD + 1:2 * HD + 1],
                                ppos_tiles[h][:mr, mt, c * 128:c * 128 + nr],
                                vv_sb[:mr, b, mt, hc],
                                start=(mt == 0), stop=(mt == MT - 1))
                        for mt in range(MT):
                            mr = _mrows(mt)
                            nc.tensor.matmul(
                                cp[:nr, ci, 0:HD + 1],
                                es[:mr, mt, c * 128:c * 128 + nr],
                                vv_sb[:mr, b, mt, hc1],
                                start=(mt == 0), stop=(mt == MT - 1))
                    # per-chunk normalizers 1/Z' (chunk 4 has 64 valid rows);
                    # evict pos to o, then o = content * (1/Z') + o
                    # (the DVE has a single PSUM read port, so the pos term
                    # must bounce through SBUF before the combine)
                    zr = zr_pool.tile([128, 3, 1], F32, tag="zr")
                    if chunks[0] == 0:
                        nc.vector.reciprocal(zr[:, 0:3, :], cp[:, 0:3, HD:HD + 1])
                        nc.vector.tensor_copy(
                            o_sb[:, b, 0:3, oh], cp[:, 0:3, HD + 1:2 * HD + 1])
                    else:
                        nc.vector.reciprocal(zr[:, 0:1, :], cp[:, 0:1, HD:HD + 1])
                        nc.vector.reciprocal(zr[:64, 1:2, :],
                                             cp[:64, 1:2, HD:HD + 1])
                        nc.vector.tensor_copy(
                            o_sb[:, b, 3, oh], cp[:, 0, HD + 1:2 * HD + 1])
                        nc.vector.tensor_copy(
                            o_sb[:64, b, 4, oh], cp[:64, 1, HD + 1:2 * HD + 1])
                    for ci, c in enumerate(chunks):
                        nr = _mrows(c)
                        nc.vector.scalar_tensor_tensor(
                            o_sb[:nr, b, c, oh], cp[:nr, ci, 0:HD],
                            zr[:nr, ci, 0:1], o_sb[:nr, b, c, oh],
                            op0=ALU.mult, op1=ALU.add)

            TERMS_QK = ((wqh_sb, xh_sb), (wql_sb, xh_sb))
            TERMS_V = ((xh_sb, wvh_sb), (xh_sb, wvl_sb), (xl_sb, wvh_sb))

            def qk_chain(fc, b):
                fs = slice(fc * 128, (fc + 1) * 128)
                tA = psA.tile([128, 512], F32, tag="A")
                tB = psA.tile([128, 512], F32, tag="A")
                nterm = len(TERMS_QK) * (KT // 2)
                for (no, nw, t, off) in ((0, 256, tA, 0), (256, 256, tA, 256),
                                         (512, 64, tB, 0)):
                    nmm = 0
                    for (w8, x8) in TERMS_QK:
                        for j in range(KT // 2):
                            nc.tensor.matmul(
                                t[:, off:off + nw],
                                w8[:, 2 * j:2 * j + 2, fs],
                                x8[:, 2 * j:2 * j + 2,
                                   b * N + no: b * N + no + nw],
                                start=(nmm == 0), stop=(nmm == nterm - 1),
                                perf_mode=DR)
                            nmm += 1
                nc.vector.tensor_copy(
                    qkT_sb[:, fc, b * N:b * N + 512], tA[:, 0:512])
                nc.vector.tensor_copy(
                    qkT_sb[:, fc, b * N + 512:(b + 1) * N], tB[:, 0:64])

            def v_chain(b, mt):
                mr = _mrows(mt)
                ms = slice(b * N + mt * 128, b * N + mt * 128 + mr)
                tA = psA.tile([128, 512], F32, tag="A")
                tB = psA.tile([128, 512], F32, tag="A")
                for (co, t, off) in ((0, tA, 0), (256, tA, 256), (512, tB, 0)):
                    nmm = 0
                    for (x8, w8) in TERMS_V:
                        for j in range(KT // 2):
                            nc.tensor.matmul(
                                t[:mr, off:off + 256],
                                x8[:, 2 * j:2 * j + 2, ms],
                                w8[:, 2 * j:2 * j + 2, co:co + 256],
                                start=(nmm == 0), stop=(nmm == 8),
                                perf_mode=DR)
                            nmm += 1
                vvr = vv_sb[:mr, b, mt, :].rearrange(
                    "p (h e) -> p h e", e=HD + 1)[:, :, 0:HD]
                nc.vector.tensor_copy(
                    vvr[:, 0:8, :],
                    tA[:mr, 0:512].rearrange("p (h e) -> p h e", e=HD))
                nc.vector.tensor_copy(
                    vvr[:, 8:12, :],
                    tB[:mr, 0:256].rearrange("p (h e) -> p h e", e=HD))

            def transpose_block(b, cc):
                # oT psums share the S-psum slots (fits: bf16 576 <= f32 576)
                p_t = psS.tile([128, N], BF16, tag="S")
                for nt in range(MT):
                    nr = _mrows(nt)
                    nc.tensor.transpose(
                        p_t[:, nt * 128:nt * 128 + nr],
                        o_sb[:nr, b, nt, cc * 128:(cc + 1) * 128],
                        ident_sb[:nr, :nr])
                nc.vector.tensor_copy(
                    oT_sb[:, cc, b * N:(b + 1) * N], p_t[:, 0:N])

            def proj(b):
                for cc in range(KT):
                    tA = psA.tile([128, 512], F32, tag="A")
                    tB = psA.tile([128, 512], F32, tag="A")
                    for k in range(KT):
                        for (no, nw, t, off) in ((0, 512, tA, 0),
                                                 (512, 64, tB, 0)):
                            nc.tensor.matmul(
                                t[:, off:off + nw],
                                wprojT_sb[:, k, cc * 128:(cc + 1) * 128],
                                oT_sb[:, k, b * N + no: b * N + no + nw],
                                start=(k == 0), stop=(k == KT - 1))
                    y1 = y_pool.tile([128, 512], F32, tag="ysb")
                    nc.scalar.copy(y1[:], tA[:, 0:512])
                    nc.sync.dma_start(
                        yT[b, cc * 128:(cc + 1) * 128, 0:512], y1[:])
                    y2 = y_pool.tile([128, 512], F32, tag="ysb")
                    nc.scalar.copy(y2[:, 0:64], tB[:, 0:64])
                    nc.sync.dma_start(
                        yT[b, cc * 128:(cc + 1) * 128, 512:576], y2[:, 0:64])

            def chains_for(Q, b):
                for fc in range(4 * Q, 4 * Q + 4):
                    qk_chain(fc, b)

            # ---- head start: quad 0's q/k chains + preps so ACT (exp)
            # starts early; its exps overlap the v projection ----
            chains_done = {(0, 0), (0, 1)}
            chains_for(0, 0)
            prepare(0)
            prepare(1)
            chains_for(0, 1)
            prepare(2)
            prepare(3)

            # ---- v[m, c'] (x-stationary, fp8 hi/lo DoubleRow) ----
            for b in range(BLOC):
                for mt in range(MT):
                    v_chain(b, mt)

            # ---- item-driven pipeline: chains just-in-time per (quad, b),
            # preps paced 1:1 with consumes so S/exp work fills PE/ACT until
            # the very end; transposes/projection interleave as head pairs
            # complete ----
            LAG = 4
            for i in range(LAG, len(items) + LAG):
                if i < len(items):
                    h, b = items[i]
                    if h % 4 == 0 and (h // 4, b) not in chains_done:
                        chains_done.add((h // 4, b))
                        chains_for(h // 4, b)
                    prepare(i)
                h_c, b_c = items[i - LAG]
                consume(i - LAG)
                if h_c % 2 == 1:
                    transpose_block(b_c, h_c // 2)
                    if h_c == H - 1:
                        proj(b_c)
            ps_cp_ctx.__exit__(None, None, None)
            ps_s_ctx.__exit__(None, None, None)
            ps_a_ctx.__exit__(None, None, None)

    nc.compile()
    return nc


_CACHE = {}


def _get_program(reps=1):
    if reps not in _CACHE:
        _CACHE[reps] = build_program(reps)
    return _CACHE[reps]


def _fp8_hilo(w):
    fp8 = ml_dtypes.float8_e4m3
    hi = w.astype(np.float32).astype(fp8)
    lo = (w.astype(np.float32) - hi.astype(np.float32)).astype(fp8)
    return hi, lo


def _qk_perm():
    """perm[newcol] = original Wqk row for the head-quad DoubleRow layout."""
    perm = np.zeros(F, dtype=np.int64)
    for h in range(H):
        P, s = divmod(h, 4)
        for d in range(HD):
            j, r = divmod(d, 32)
            perm[(4 * P + j) * 128 + 32 * s + r] = h * HD + d
            perm[(4 * P + 2 + j) * 128 + 32 * s + r] = C + h * HD + d
    return perm


def _host_prep(x, Wqk, Wv, Wproj, bproj, Wpos, bpos, gating):
    bf = ml_dtypes.bfloat16
    g = 1.0 / (1.0 + np.exp(-gating.astype(np.float64)))          # [H]

    s = int(N ** 0.5)
    ind = np.arange(s)[None, :] - np.arange(s)[:, None]
    indx = np.tile(ind, (s, s)).astype(np.float64)
    indy = np.repeat(np.repeat(ind, s, axis=0), s, axis=1).astype(np.float64)
    indd = indx ** 2 + indy ** 2
    rel = np.stack([indx, indy, indd], axis=-1)                    # [N, N, 3]
    pos_logits = np.einsum("nmt,ht->hnm", rel, Wpos.astype(np.float64))
    pos_logits += bpos.astype(np.float64)[:, None, None]
    pos_logits -= pos_logits.max(axis=-1, keepdims=True)
    e = np.exp(pos_logits)
    pos = e / e.sum(axis=-1, keepdims=True)                        # [H, n, m]
    # prescaled by g/WS to undo the WS factor the fp8 v projection carries
    ppos_w = (g[:, None, None] / WS) * pos
    pposT = np.ascontiguousarray(ppos_w.transpose(0, 2, 1)).astype(bf)

    wq_perm = np.ascontiguousarray(Wqk[_qk_perm(), :].T)           # [C, F]
    wqh, wql = _fp8_hilo(wq_perm * WSQ)
    wvh, wvl = _fp8_hilo(np.ascontiguousarray(Wv.T) * WS)
    # Z' column value WS/(1-g): combine's recip then yields (1-g)/(WS*Z)
    # so content * that + pos is exact
    vcol = (WS / (1.0 - g)).astype(np.float32)                     # [H]
    vones = np.broadcast_to(vcol[None, None, :],
                            (BLOC * MT, 128, H)).transpose(1, 0, 2)
    vones = np.ascontiguousarray(vones.reshape(128, BLOC * MT * H))

    common = {
        "wqh": wqh, "wql": wql, "wvh": wvh, "wvl": wvl,
        "wprojT": np.ascontiguousarray(Wproj.T).astype(bf),
        "ident": np.eye(128, dtype=np.float32).astype(bf),
        "vones": vones,
        "pposT": pposT,
    }
    in_maps = []
    for i in range(NCORES):
        xc = x[i * BLOC:(i + 1) * BLOC]                            # [2, 576, 768]
        xTl = np.ascontiguousarray(
            xc.transpose(2, 0, 1).reshape(C, NT)).astype(np.float32)
        xhl, xll = _fp8_hilo(xTl)
        in_maps.append({"xh": xhl, "xl": xll, **common})
    return in_maps


def kernel(x, Wqk, Wv, Wproj, bproj, Wpos, bpos, gating):
    x = np.asarray(x, dtype=np.float32)
    in_maps = _host_prep(np.asarray(x, np.float32), np.asarray(Wqk, np.float32),
                         np.asarray(Wv, np.float32), np.asarray(Wproj, np.float32),
                         np.asarray(bproj, np.float32), np.asarray(Wpos, np.float32),
                         np.asarray(bpos, np.float32), np.asarray(gating, np.float32))
    nc = _get_program(reps=1)
    res = run_bass_kernel_spmd(nc, in_maps, list(range(NCORES)))
    outs = []
    for i in range(NCORES):
        yTl = res.results[i]["yT"]                                 # [2, 768, 576]
        outs.append(yTl.transpose(0, 2, 1))                        # [2, 576, 768]
    out = np.concatenate(outs, axis=0).astype(np.float32)
    out += np.asarray(bproj, np.float32)[None, None, :]
    return np.ascontiguousarray(out)


# revision 10
# speedup vs baseline: 1.3884x; 1.2300x over previous
"""GPSA (gated positional self-attention) Trainium2 Bass kernel.

Problem: B=16, N=576, C=768, H=12 heads, hd=64.
  qk = x @ Wqk.T -> q,k [B,H,N,64]
  patch = softmax(q k^T / 8), pos = softmax(rel @ Wpos + bpos)  [H,N,N]
  attn = (1-sig(g))*patch + sig(g)*pos  (sums to 1 -> final renorm is identity)
  out = (attn @ v) @ Wproj.T + bproj

Sharding: pure data-parallel over batch, 2 batches per core, no collectives.

Cost-model-driven layout (matmul cost = out-free-size x k-instrs; fp8
DoubleRow halves the per-instr cost by packing 2 contraction slabs):
  qk projection: fp8 2-term (Wqk hi/lo at WSQ=32, x hi only) -> psum holds
  WSQ*q; evicted directly to fp8 q/k (|WSQ*q| < 240).
  Head features are host-permuted so head h (quad P=h//4, slot s=h%4) has
  q features as [32 partitions at 32s, 2 planes = chunks 4P/4P+1] and k in
  chunks 4P+2/4P+3.  S^T then runs as a DoubleRow fp8 matmul (0.5 cyc/row):
  stationary k [32, 2, m], moving q [32, 2, n], tile_position (32s, 0).
  v projection: fp8 3-term hi/lo at WS=64 (v feeds the output linearly, so
  it needs the extra term; q/k errors wash out through softmax).
  exp on ACT (scale 1/(8*WSQ^2); max-subtraction skipped).
  PV runs "flipped" in bf16: es/ppos [m, n-chunk<=128] stationary, v moving
  [m, 65] -> out [n-chunk, 65].  Column 64 of the content output is the
  softmax normalizer Z' = WS/(1-g) * Z via a constant column appended to v.
  The combine is a single fused DVE op per chunk:
  o[n,c] = content * recip(Z'col) + pos  (both read straight from PSUM).
  o[n,c] is PE-transposed (identity trick) into oT[c,n] and projected
  W-stationary in bf16.
PSUM: chains (qk/V/proj) use 3x1-bank [128,512] tiles, S^T 2x2-bank
[128,576], PV one 1-bank [128,3,129] two-pass tile -> exactly 8 banks.
DMA: one large transfer per tensor region (HWDGE generation is 625ns per
DMA, serialized), ordered so the first qk chains / v chains / ppos arrive
just in time.  The host precomputes sigmoid(gating), the positional softmax
(prescaled by g/WS), weight transposes, permutations and fp8 splits.
"""

import numpy as np
import ml_dtypes

from contextlib import ExitStack

import concourse.tile as tile
from concourse import bacc, mybir
from concourse.bass_utils import run_bass_kernel_spmd

BF16 = mybir.dt.bfloat16
F32 = mybir.dt.float32
FP8 = mybir.dt.float8e4
AF = mybir.ActivationFunctionType
ALU = mybir.AluOpType
DR = mybir.MatmulPerfMode.DoubleRow

B, N, C, H = 16, 576, 768, 12
HD = C // H                      # 64
NCORES = 8
BLOC = B // NCORES               # batches per core
NT = BLOC * N                    # tokens per core (1152)
F = 2 * C                        # fused qk features (1536)
KT = C // 128                    # 6 contraction tiles over C
FT = F // 128                    # 12 feature tiles of qkT
MT = (N + 127) // 128            # 5 m-tiles per batch (last has 64 rows)
NQ = 3                           # head quads
WS = 64.0                        # host pre-scale on Wv for fp8 hi/lo
WSQ = 32.0                       # host pre-scale on Wqk (fp8 q/k fits 240)
EXPSCALE = (HD ** -0.5) / (WSQ * WSQ)


def _mrows(mt):
    return min(128, N - mt * 128)


def build_program(reps=1):
    nc = bacc.Bacc("TRN2", target_bir_lowering=False, debug=False,
                   num_devices=NCORES)

    xh = nc.declare_dram_parameter("xh", [C, NT], FP8, isOutput=False)
    xl = nc.declare_dram_parameter("xl", [C, NT], FP8, isOutput=False)
    wqh = nc.declare_dram_parameter("wqh", [C, F], FP8, isOutput=False)
    wql = nc.declare_dram_parameter("wql", [C, F], FP8, isOutput=False)
    wvh = nc.declare_dram_parameter("wvh", [C, C], FP8, isOutput=False)
    wvl = nc.declare_dram_parameter("wvl", [C, C], FP8, isOutput=False)
    wprojT = nc.declare_dram_parameter("wprojT", [C, C], BF16, isOutput=False)
    ident = nc.declare_dram_parameter("ident", [128, 128], BF16, isOutput=False)
    vones = nc.declare_dram_parameter("vones", [128, BLOC * MT * H], F32,
                                      isOutput=False)
    pposT = nc.declare_dram_parameter("pposT", [H, N, N], BF16, isOutput=False)
    yT = nc.declare_dram_parameter("yT", [BLOC, C, N], F32, isOutput=True)

    with tile.TileContext(nc) as tc, ExitStack() as ctx:
        sbw = ctx.enter_context(tc.tile_pool(name="sbw", bufs=1))
        sbact = ctx.enter_context(tc.tile_pool(name="sbact", bufs=1))
        ppos_pool = ctx.enter_context(tc.tile_pool(name="ppos", bufs=5))
        es_pool = ctx.enter_context(tc.tile_pool(name="es", bufs=6))
        zr_pool = ctx.enter_context(tc.tile_pool(name="zr", bufs=4))
        y_pool = ctx.enter_context(tc.tile_pool(name="ysb", bufs=4))

        # ---- weights / constants.  Few large DMAs (HWDGE gen is 625ns
        # each, serialized) ordered so the first qk chains start ASAP. ----
        xh_sb = sbw.tile([128, KT, NT], FP8)
        xl_sb = sbw.tile([128, KT, NT], FP8)
        wqh_sb = sbw.tile([128, KT, F], FP8)
        wql_sb = sbw.tile([128, KT, F], FP8)
        wvh_sb = sbw.tile([128, KT, C], FP8)
        wvl_sb = sbw.tile([128, KT, C], FP8)
        wprojT_sb = sbw.tile([128, KT, C], BF16)
        ident_sb = sbw.tile([128, 128], BF16)
        vones_sb = sbw.tile([128, BLOC * MT * H], F32)

        def _r(dram, cols):          # [rows, cols] -> [128, KT', cols]
            return dram.rearrange("(k p) n -> p k n", p=128)

        xh_r, xl_r = _r(xh, NT), _r(xl, NT)
        wqh_r, wql_r = _r(wqh, F), _r(wql, F)

        def load_wq_quad(q):
            fs = slice(512 * q, 512 * (q + 1))
            nc.sync.dma_start(wqh_sb[:, :, fs], wqh_r[:, :, fs])
            nc.sync.dma_start(wql_sb[:, :, fs], wql_r[:, :, fs])

        # batch-0 x and v weights first (v chains fill the PE from ~3us),
        # then quad-0 qk weights and the tiny constants; the rest of the
        # weights stream behind the first ppos loads (emitted in-loop).
        nc.sync.dma_start(xh_sb[:, :, 0:N], xh_r[:, :, 0:N])
        nc.sync.dma_start(wvh_sb[:, :, :], _r(wvh, C)[:, :, :])
        nc.sync.dma_start(wvl_sb[:, :, :], _r(wvl, C)[:, :, :])
        nc.sync.dma_start(xl_sb[:, :, 0:N], xl_r[:, :, 0:N])
        load_wq_quad(0)
        nc.sync.dma_start(ident_sb[:], ident[:, :])
        nc.sync.dma_start(vones_sb[:], vones[:, :])

        def load_rest():
            nc.sync.dma_start(xh_sb[:, :, N:NT], xh_r[:, :, N:NT])
            nc.sync.dma_start(xl_sb[:, :, N:NT], xl_r[:, :, N:NT])
            load_wq_quad(1)
            load_wq_quad(2)
            nc.sync.dma_start(wprojT_sb[:, :, :],
                              wprojT.rearrange("(k p) n -> p k n", p=128)[:, :, :])

        loaded_rest = [False]

        for _ in range(reps):
            ps_a_ctx = tc.tile_pool(name="ps_a", bufs=3, space="PSUM")
            psA = ps_a_ctx.__enter__()
            ps_s_ctx = tc.tile_pool(name="ps_s", bufs=2, space="PSUM")
            psS = ps_s_ctx.__enter__()
            ps_cp_ctx = tc.tile_pool(name="ps_cp", bufs=1, space="PSUM")
            psCP = ps_cp_ctx.__enter__()

            qkT_sb = sbact.tile([128, FT, NT], FP8)
            # v layout [tokens, 12*65]: per head 64 v-cols + a constant column
            # WS/(1-g_h) making the content-PV's 65th output column the
            # prescaled softmax normalizer Z'
            vv_sb = sbact.tile([128, BLOC, MT, H * (HD + 1)], BF16)
            ones_cols = vv_sb[:, :, :, :].rearrange(
                "p b m (h e) -> p b m h e", e=HD + 1)[:, :, :, :, HD:HD + 1]
            vones_src = vones_sb[:, :].rearrange(
                "p (b m h) -> p b m h", b=BLOC, m=MT)[:, :, :, :, None]
            nc.vector.tensor_copy(ones_cols, vones_src)
            # o[n, c] per batch: [n-part within chunk, b, n-chunk, (h d)]
            o_sb = sbact.tile([128, BLOC, MT, C], BF16)
            oT_sb = sbact.tile([128, KT, NT], BF16)

            items = [(4 * Q + hs, b)
                     for Q in range(NQ)
                     for b in range(BLOC)
                     for hs in range(4)]
            ppos_tiles = {}
            es_tiles = {}

            def dma_ppos(hp):
                for hsub in range(2):
                    h = 2 * hp + hsub
                    pt = ppos_pool.tile([128, MT, N], BF16, tag="ppos")
                    pr = pposT[h, 0:512, :].rearrange("(m p) n -> p m n", p=128)
                    nc.sync.dma_start(pt[:, 0:4, :], pr)
                    nc.sync.dma_start(pt[0:64, 4, :],
                                      pposT[h, 512:576, :])
                    ppos_tiles[h] = pt

            def prepare(i):
                h, b = items[i]
                if b == 0 and h % 2 == 0:
                    dma_ppos(h // 2)
                P, s = h // 4, h % 4
                rows = slice(32 * s, 32 * s + 32)
                qc = slice(4 * P, 4 * P + 2)
                kc = slice(4 * P + 2, 4 * P + 4)
                es = es_pool.tile([128, MT, N], BF16, tag="es")
                for mt in range(MT):
                    mr = _mrows(mt)
                    ms = slice(b * N + mt * 128, b * N + mt * 128 + mr)
                    p_s = psS.tile([128, N], F32, tag="S")
                    for (no, nw) in ((0, 512), (512, 64)):
                        nc.tensor.matmul(
                            p_s[:mr, no:no + nw],
                            qkT_sb[rows, kc, ms],
                            qkT_sb[rows, qc, b * N + no: b * N + no + nw],
                            start=True, stop=True, perf_mode=DR,
                            tile_position=(32 * s, 0))
                    nc.scalar.activation(
                        es[:mr, mt, :], p_s[:mr, 0:N], AF.Exp, scale=EXPSCALE)
                es_tiles[i] = es

            def pos_pv(i):
                # pos half of PV: its own pipeline stage so the content pass
                # below owns the psCP bank for a single round-trip per item.
                # One [128, 5, 64] psum (rides a psA slot) + one DVE evict.
                h, b = items[i]
                hc = slice(h * (HD + 1), h * (HD + 1) + HD)
                oh = slice(h * HD, (h + 1) * HD)
                pp = psA.tile([128, MT, HD], F32, tag="A")
                for c in range(MT):
                    nr = _mrows(c)
                    for mt in range(MT):
                        mr = _mrows(mt)
                        nc.tensor.matmul(
                            pp[:nr, c, :],
                            ppos_tiles[h][:mr, mt, c * 128:c * 128 + nr],
                            vv_sb[:mr, b, mt, hc],
                            start=(mt == 0), stop=(mt == MT - 1))
                nc.vector.tensor_copy(o_sb[:, b, 0:4, oh], pp[:, 0:4, :])
                nc.vector.tensor_copy(o_sb[:64, b, 4, oh], pp[:64, 4, :])

            def consume(i):
                h, b = items[i]
                es = es_tiles.pop(i)
                hc1 = slice(h * (HD + 1), (h + 1) * (HD + 1))
                oh = slice(h * HD, (h + 1) * HD)
                cp = psCP.tile([128, MT, HD + 1], F32, tag="cp")
                for c in range(MT):
                    nr = _mrows(c)
                    for mt in range(MT):
                        mr = _mrows(mt)
                        nc.tensor.matmul(
                            cp[:nr, c, :],
                            es[:mr, mt, c * 128:c * 128 + nr],
                            vv_sb[:mr, b, mt, hc1],
                            start=(mt == 0), stop=(mt == MT - 1))
                # per-chunk normalizers 1/Z' (chunk 4 has 64 valid rows);
                # o = content * (1/Z') + o(pos)  — pos was pre-placed in o_sb
                # by pos_pv, so the stt has a single PSUM operand.
                zr = zr_pool.tile([128, MT, 1], F32, tag="zr")
                nc.vector.reciprocal(zr[:, 0:4, :], cp[:, 0:4, HD:HD + 1])
                nc.vector.reciprocal(zr[:64, 4:5, :], cp[:64, 4:5, HD:HD + 1])
                for c in range(MT):
                    nr = _mrows(c)
                    nc.vector.scalar_tensor_tensor(
                        o_sb[:nr, b, c, oh], cp[:nr, c, 0:HD],
                        zr[:nr, c, 0:1], o_sb[:nr, b, c, oh],
                        op0=ALU.mult, op1=ALU.add)

            TERMS_QK = ((wqh_sb, xh_sb), (wql_sb, xh_sb))
            TERMS_V = ((xh_sb, wvh_sb), (xh_sb, wvl_sb), (xl_sb, wvh_sb))

            def qk_chain(fc, b):
                fs = slice(fc * 128, (fc + 1) * 128)
                tA = psA.tile([128, 512], F32, tag="A")
                tB = psA.tile([128, 512], F32, tag="A")
                nterm = len(TERMS_QK) * (KT // 2)
                for (no, nw, t, off) in ((0, 256, tA, 0), (256, 256, tA, 256),
                                         (512, 64, tB, 0)):
                    nmm = 0
                    for (w8, x8) in TERMS_QK:
                        for j in range(KT // 2):
                            nc.tensor.matmul(
                                t[:, off:off + nw],
                                w8[:, 2 * j:2 * j + 2, fs],
                                x8[:, 2 * j:2 * j + 2,
                                   b * N + no: b * N + no + nw],
                                start=(nmm == 0), stop=(nmm == nterm - 1),
                                perf_mode=DR)
                            nmm += 1
                nc.vector.tensor_copy(
                    qkT_sb[:, fc, b * N:b * N + 512], tA[:, 0:512])
                nc.vector.tensor_copy(
                    qkT_sb[:, fc, b * N + 512:(b + 1) * N], tB[:, 0:64])

            def v_chain(b, mt):
                mr = _mrows(mt)
                ms = slice(b * N + mt * 128, b * N + mt * 128 + mr)
                tA = psA.tile([128, 512], F32, tag="A")
                tB = psA.tile([128, 512], F32, tag="A")
                for (co, t, off) in ((0, tA, 0), (256, tA, 256), (512, tB, 0)):
                    nmm = 0
                    for (x8, w8) in TERMS_V:
                        for j in range(KT // 2):
                            nc.tensor.matmul(
                                t[:mr, off:off + 256],
                                x8[:, 2 * j:2 * j + 2, ms],
                                w8[:, 2 * j:2 * j + 2, co:co + 256],
                                start=(nmm == 0), stop=(nmm == 8),
                                perf_mode=DR)
                            nmm += 1
                vvr = vv_sb[:mr, b, mt, :].rearrange(
                    "p (h e) -> p h e", e=HD + 1)[:, :, 0:HD]
                nc.vector.tensor_copy(
                    vvr[:, 0:8, :],
                    tA[:mr, 0:512].rearrange("p (h e) -> p h e", e=HD))
                nc.vector.tensor_copy(
                    vvr[:, 8:12, :],
                    tB[:mr, 0:256].rearrange("p (h e) -> p h e", e=HD))

            def transpose_block(b, cc):
                # oT psums share the S-psum slots (fits: bf16 576 <= f32 576)
                p_t = psS.tile([128, N], BF16, tag="S")
                for nt in range(MT):
                    nr = _mrows(nt)
                    nc.tensor.transpose(
                        p_t[:, nt * 128:nt * 128 + nr],
                        o_sb[:nr, b, nt, cc * 128:(cc + 1) * 128],
                        ident_sb[:nr, :nr])
                nc.vector.tensor_copy(
                    oT_sb[:, cc, b * N:(b + 1) * N], p_t[:, 0:N])

            def proj(b):
                for cc in range(KT):
                    tA = psA.tile([128, 512], F32, tag="A")
                    tB = psA.tile([128, 512], F32, tag="A")
                    for k in range(KT):
                        for (no, nw, t, off) in ((0, 512, tA, 0),
                                                 (512, 64, tB, 0)):
                            nc.tensor.matmul(
                                t[:, off:off + nw],
                                wprojT_sb[:, k, cc * 128:(cc + 1) * 128],
                                oT_sb[:, k, b * N + no: b * N + no + nw],
                                start=(k == 0), stop=(k == KT - 1))
                    y1 = y_pool.tile([128, 512], F32, tag="ysb")
                    nc.scalar.copy(y1[:], tA[:, 0:512])
                    nc.sync.dma_start(
                        yT[b, cc * 128:(cc + 1) * 128, 0:512], y1[:])
                    y2 = y_pool.tile([128, 512], F32, tag="ysb")
                    nc.scalar.copy(y2[:, 0:64], tB[:, 0:64])
                    nc.sync.dma_start(
                        yT[b, cc * 128:(cc + 1) * 128, 512:576], y2[:, 0:64])

            def chains_for(Q, b):
                for fc in range(4 * Q, 4 * Q + 4):
                    qk_chain(fc, b)

            # ---- head start: v chains b0 first (x/wv arrive first), then
            # quad-0 q/k chains + pos/prepare so PE fills from ~3us and ACT
            # (exp) starts early ----
            dma_ppos(0)
            dma_ppos(1)
            if not loaded_rest[0]:
                loaded_rest[0] = True
                load_rest()
            for mt in range(MT):
                v_chain(0, mt)
            chains_for(0, 0)
            pos_pv(0)
            prepare(0)
            pos_pv(1)
            prepare(1)
            for mt in range(MT):
                v_chain(1, mt)
            pos_pv(2)
            prepare(2)
            pos_pv(3)
            prepare(3)
            chains_for(0, 1)
            chains_done = {(0, 0), (0, 1)}

            # ---- item-driven pipeline: chains just-in-time per (quad, b);
            # pos_pv leads prepare; consume lags by 4 (es pool depth) and
            # transposes by one more so the PE never waits on the DVE
            # combine; projection fires as each batch's last head lands ----
            LAG = 4
            TLAG = LAG + 1
            next_hp = 2
            for i in range(LAG, len(items) + TLAG):
                if i < len(items):
                    h, b = items[i]
                    if h % 4 == 0 and (h // 4, b) not in chains_done:
                        chains_done.add((h // 4, b))
                        chains_for(h // 4, b)
                    if b == 1 and h % 2 == 0 and next_hp < H // 2:
                        dma_ppos(next_hp)
                        next_hp += 1
                    pos_pv(i)
                    prepare(i)
                if LAG <= i < len(items) + LAG:
                    consume(i - LAG)
                if i >= TLAG:
                    h_c, b_c = items[i - TLAG]
                    if h_c % 2 == 1:
                        transpose_block(b_c, h_c // 2)
                        if h_c == H - 1:
                            proj(b_c)
            ps_cp_ctx.__exit__(None, None, None)
            ps_s_ctx.__exit__(None, None, None)
            ps_a_ctx.__exit__(None, None, None)

    nc.compile()
    return nc


_CACHE = {}


def _get_program(reps=1):
    if reps not in _CACHE:
        _CACHE[reps] = build_program(reps)
    return _CACHE[reps]


def _fp8_hilo(w):
    fp8 = ml_dtypes.float8_e4m3
    hi = w.astype(np.float32).astype(fp8)
    lo = (w.astype(np.float32) - hi.astype(np.float32)).astype(fp8)
    return hi, lo


def _qk_perm():
    """perm[newcol] = original Wqk row for the head-quad DoubleRow layout."""
    perm = np.zeros(F, dtype=np.int64)
    for h in range(H):
        P, s = divmod(h, 4)
        for d in range(HD):
            j, r = divmod(d, 32)
            perm[(4 * P + j) * 128 + 32 * s + r] = h * HD + d
            perm[(4 * P + 2 + j) * 128 + 32 * s + r] = C + h * HD + d
    return perm


def _host_prep(x, Wqk, Wv, Wproj, bproj, Wpos, bpos, gating):
    bf = ml_dtypes.bfloat16
    g = 1.0 / (1.0 + np.exp(-gating.astype(np.float64)))          # [H]

    s = int(N ** 0.5)
    ind = np.arange(s)[None, :] - np.arange(s)[:, None]
    indx = np.tile(ind, (s, s)).astype(np.float64)
    indy = np.repeat(np.repeat(ind, s, axis=0), s, axis=1).astype(np.float64)
    indd = indx ** 2 + indy ** 2
    rel = np.stack([indx, indy, indd], axis=-1)                    # [N, N, 3]
    pos_logits = np.einsum("nmt,ht->hnm", rel, Wpos.astype(np.float64))
    pos_logits += bpos.astype(np.float64)[:, None, None]
    pos_logits -= pos_logits.max(axis=-1, keepdims=True)
    e = np.exp(pos_logits)
    pos = e / e.sum(axis=-1, keepdims=True)                        # [H, n, m]
    # prescaled by g/WS to undo the WS factor the fp8 v projection carries
    ppos_w = (g[:, None, None] / WS) * pos
    pposT = np.ascontiguousarray(ppos_w.transpose(0, 2, 1)).astype(bf)

    wq_perm = np.ascontiguousarray(Wqk[_qk_perm(), :].T)           # [C, F]
    wqh, wql = _fp8_hilo(wq_perm * WSQ)
    wvh, wvl = _fp8_hilo(np.ascontiguousarray(Wv.T) * WS)
    # Z' column value WS/(1-g): combine's recip then yields (1-g)/(WS*Z)
    # so content * that + pos is exact
    vcol = (WS / (1.0 - g)).astype(np.float32)                     # [H]
    vones = np.broadcast_to(vcol[None, None, :],
                            (BLOC * MT, 128, H)).transpose(1, 0, 2)
    vones = np.ascontiguousarray(vones.reshape(128, BLOC * MT * H))

    common = {
        "wqh": wqh, "wql": wql, "wvh": wvh, "wvl": wvl,
        "wprojT": np.ascontiguousarray(Wproj.T).astype(bf),
        "ident": np.eye(128, dtype=np.float32).astype(bf),
        "vones": vones,
        "pposT": pposT,
    }
    in_maps = []
    for i in range(NCORES):
        xc = x[i * BLOC:(i + 1) * BLOC]                            # [2, 576, 768]
        xTl = np.ascontiguousarray(
            xc.transpose(2, 0, 1).reshape(C, NT)).astype(np.float32)
        xhl, xll = _fp8_hilo(xTl)
        in_maps.append({"xh": xhl, "xl": xll, **common})
    return in_maps


def kernel(x, Wqk, Wv, Wproj, bproj, Wpos, bpos, gating):
    x = np.asarray(x, dtype=np.float32)
    in_maps = _host_prep(np.asarray(x, np.float32), np.asarray(Wqk, np.float32),
                         np.asarray(Wv, np.float32), np.asarray(Wproj, np.float32),
                         np.asarray(bproj, np.float32), np.asarray(Wpos, np.float32),
                         np.asarray(bpos, np.float32), np.asarray(gating, np.float32))
    nc = _get_program(reps=1)
    res = run_bass_kernel_spmd(nc, in_maps, list(range(NCORES)))
    outs = []
    for i in range(NCORES):
        yTl = res.results[i]["yT"]                                 # [2, 768, 576]
        outs.append(yTl.transpose(0, 2, 1))                        # [2, 576, 768]
    out = np.concatenate(outs, axis=0).astype(np.float32)
    out += np.asarray(bproj, np.float32)[None, None, :]
    return np.ascontiguousarray(out)
